# revision 10
# baseline (speedup 1.0000x reference)
"""Trainium2 Bass kernel for nn_CausalAttention_5815385719336.

Dual-softmax attention: out = softmax(-QK^T/8) V Wo^T (+bias folds),
out_comp = softmax(+QK^T/8) V Wo^T.  B=2, S=2048, D=1024, H=16, DK=64.

Sharding (8 cores): Megatron-style head parallel.  Core c owns heads
(2c, 2c+1) = output dims [128c, 128c+128) of the QKV projections.  Each
core computes its head slice of Q/K/V for both batches, the full [S,S]
attention for its 4 (b, head) units (both softmax branches), and a
partial output projection o_slice @ Wo_slice^T.  The host sums the 8
partial outputs and adds the bias fold (bv @ Wo^T + bo).

v3 engine-balanced pipeline:
  - Prologue runs ALL Q/K/V projections as dense back-to-back matmul
    chains (PE ramps to the full p-state clock; ACT has nothing to do
    that early anyway).
  - exp(-s/8) always comes from ACT (bf16); exp(+s/8) is either a
    second ACT Exp (a tunable subset of kt steps) or a GPSIMD/Pool
    elementwise divide ones/exp(-s) (exp(+x) = 1/exp(-x)), splitting
    the 33M-element exponential work across the scalar engine and the
    otherwise-idle Pool engine.
  - PSUM: flex pool 2x[128,1024] (scores / denom-broadcast / outproj)
    + acc pool 2x[128,1024] (PV+- accumulators with a ones-row
    denominator) = exactly 8 banks.
  - Softmax denominators: ones column in V -> acc row 64; ACT
    Ln/Exp(-x) reciprocal; PE ones-matmul broadcast; DVE copy+mul.
  - Output projection chunks interleave into the next unit's kt loop
    as background PE work; results cast on DVE and DMAd out bf16.
"""

import numpy as np
import ml_dtypes

B, S, D, H, DK = 2, 2048, 1024, 16, 64
NCORES = 8
HPC = H // NCORES          # heads per core = 2
DSL = HPC * DK             # d-slice per core = 128
P = 128
BF16 = ml_dtypes.bfloat16

# kt steps (of 16) whose exp(+s/8) is a second ACT Exp; the rest are
# GPSIMD divides of the exp(-s/8) tile
ACT_EP_KT = frozenset((2, 5, 8, 11, 14))

_compiled = {}


def _install_drain_split():
    """walrus in this container rejects >1 sync wait on the Tile tail
    Drain; split extra waits into standalone wait_ge instructions."""
    import concourse.tile as tile
    from concourse.vector_clock import ScopedClock

    if getattr(tile.TileContext, "_drain_split_installed", False):
        return

    def _drain_and_barrier(self, tick_clock, wait_clock):
        nc = self.nc
        drain_inst = nc.sync.drain()
        wait_clock.add_sem_waits(
            drain_inst.ins, ScopedClock({None: tick_clock.global_clock})
        )
        si = drain_inst.ins.sync_info
        if si is not None and si.on_wait and len(si.on_wait) > 1:
            waits = list(si.on_wait)
            handles = {h.num: h for h in self.sems.allocated().values()}
            si.on_wait = waits[:1]
            for w in waits[1:]:
                assert w.wait_mode == "sem-ge-imm", w.wait_mode
                nc.sync.wait_ge(handles[w.id], w.wait_value)
        nc.all_engine_barrier()
        popped = nc._tile_sem_poison_stack.pop()
        assert popped is self._sem_poison
        nc.clear_and_free_semaphores(list(self.sems.allocated().values()))
        nc.all_engine_barrier()

    tile.TileContext._drain_and_barrier = _drain_and_barrier
    tile.TileContext._drain_split_installed = True


def _split_sync_waits(nc, max_waits=1):
    """walrus in this container has a small per-instruction sync-wait
    capacity.  Hoist excess waits onto standalone EventSemaphore
    instructions inserted just before the owner on the same engine —
    program order within an engine keeps the semantics identical."""
    from concourse import mybir

    n = 0
    for bb in nc.main_func.blocks:
        out = []
        for ins in bb.instructions:
            si = ins.sync_info
            if si is not None and si.on_wait and len(si.on_wait) > max_waits:
                waits = list(si.on_wait)
                for w in waits[:-max_waits]:
                    wi = mybir.InstEventSemaphore(name=f"W-split-{n}", ins=[], outs=[])
                    n += 1
                    wi.engine = ins.engine
                    wi.sync_info = mybir.SyncInfo(on_wait=[w], on_update=[])
                    out.append(wi)
                si.on_wait = waits[-max_waits:]
            out.append(ins)
        if n:
            bb.instructions = out


def _build():
    import concourse.bass as bass
    import concourse.tile as tile
    from concourse import mybir

    _install_drain_split()

    f32 = mybir.dt.float32
    f32r = mybir.dt.float32r
    bf16 = mybir.dt.bfloat16
    Exp = mybir.ActivationFunctionType.Exp
    Log = mybir.ActivationFunctionType.Ln
    NT = B * S                      # 4096 tokens
    ET = D // P                     # 8 e-tiles

    nc = bass.Bass()
    xt_d = nc.declare_dram_parameter("xt", [P, ET, NT], bf16, isOutput=False)
    wq_d = nc.declare_dram_parameter("wq", [P, ET, DSL], bf16, isOutput=False)
    wk_d = nc.declare_dram_parameter("wk", [P, ET, DSL], bf16, isOutput=False)
    wv_d = nc.declare_dram_parameter("wv", [P, ET, DSL], bf16, isOutput=False)
    wo_d = nc.declare_dram_parameter("wo", [P, D], bf16, isOutput=False)
    bq_d = nc.declare_dram_parameter("bq", [P, 1], f32, isOutput=False)
    bk_d = nc.declare_dram_parameter("bk", [P, 1], f32, isOutput=False)
    out_d = nc.declare_dram_parameter("out", [2, B, S, D], bf16, isOutput=True)

    KT = S // P                     # 16 k-tiles per batch
    TT = S // P                     # 16 token-tiles per batch
    QC = 2                          # q chunks per batch
    QW = S // QC                    # 1024
    XC = S // 512                   # 4 x-chunks (512 tokens) per batch

    with tile.TileContext(nc) as tc:
        with (
            tc.tile_pool(name="singles", bufs=1) as singles,
            tc.tile_pool(name="xst", bufs=3) as xst,
            tc.tile_pool(name="emp", bufs=4) as emp,
            tc.tile_pool(name="epp", bufs=4) as epp,
            tc.tile_pool(name="rcpp", bufs=2) as rcpp,
            tc.tile_pool(name="oTup", bufs=4) as oTup,
            tc.tile_pool(name="outp", bufs=3) as outp,
            # 8 PSUM banks: ps_flex 2x[128,1024] = 4 (scores, denom
            # broadcast, projections, outproj), ps_acc 2x = 4 (the two
            # PV accumulators of the active unit).
            tc.tile_pool(name="ps_flex", bufs=2, space="PSUM") as ps_flex,
            tc.tile_pool(name="ps_acc", bufs=2, space="PSUM") as ps_acc,
        ):
            wq = singles.tile([P, ET, DSL], bf16)
            nc.sync.dma_start(wq[:], wq_d[:])
            wk = singles.tile([P, ET, DSL], bf16)
            nc.sync.dma_start(wk[:], wk_d[:])
            wv = singles.tile([P, ET, DSL], bf16)
            nc.sync.dma_start(wv[:], wv_d[:])
            wo = singles.tile([P, D], bf16)
            nc.sync.dma_start(wo[:], wo_d[:])
            bq = singles.tile([P, 1], f32)
            nc.sync.dma_start(bq[:], bq_d[:])
            bk = singles.tile([P, 1], f32)
            nc.sync.dma_start(bk[:], bk_d[:])
            ones_sb = singles.tile([P, P], bf16)
            nc.vector.memset(ones_sb[:], 1.0)
            ones_full = singles.tile([P, QW], bf16)
            nc.vector.memset(ones_full[:], 1.0)

            # persistent per-batch projection outputs
            qT = [singles.tile([P, S], bf16, name=f"qT{b}") for b in range(B)]
            kT = [singles.tile([P, S], bf16, name=f"kT{b}") for b in range(B)]
            vt = [singles.tile([P, TT, 130], bf16, name=f"vt{b}")
                  for b in range(B)]
            for b in range(B):
                nc.vector.memset(vt[b][:, :, 64], 1.0)
                nc.vector.memset(vt[b][:, :, 129], 1.0)
            # oTs[br][b]: normalized o^T slices, bf16
            oTs = [[singles.tile([P, S], bf16, name=f"oTs{br}_{b}")
                    for b in range(B)] for br in range(2)]

            # ---------- prologue: all projections, dense PE chains ----------
            for b in range(B):
                t0 = b * S
                for xc in range(XC):
                    xtile = xst.tile([P, ET, 512], bf16, tag="xt",
                                     name=f"xt_{b}_{xc}")
                    nc.sync.dma_start(
                        xtile[:],
                        xt_d[:, :, t0 + xc * 512 : t0 + (xc + 1) * 512],
                    )
                    # Q and K chains share one flex tile (two 512 halves)
                    pqk = ps_flex.tile([P, 1024], f32, tag="flex",
                                       name=f"pqk_{b}_{xc}")
                    for et in range(ET):
                        nc.tensor.matmul(
                            pqk[:, 0:512], wq[:, et, :], xtile[:, et, :],
                            start=(et == 0), stop=(et == ET - 1),
                        )
                    for et in range(ET):
                        nc.tensor.matmul(
                            pqk[:, 512:1024], wk[:, et, :], xtile[:, et, :],
                            start=(et == 0), stop=(et == ET - 1),
                        )
                    nc.vector.tensor_scalar_add(
                        qT[b][:, xc * 512 : (xc + 1) * 512], pqk[:, 0:512], bq
                    )
                    nc.vector.tensor_scalar_add(
                        kT[b][:, xc * 512 : (xc + 1) * 512], pqk[:, 512:1024],
                        bk,
                    )
                    # V chains: 4 token-tiles per x chunk, tokens on the
                    # out partitions (stationary = x slice)
                    pv = ps_acc.tile([P, 1024], f32, tag="acc",
                                     name=f"pv_{b}_{xc}")
                    for vtt in range(4):
                        tt = xc * 4 + vtt
                        sl = pv[:, vtt * 256 : vtt * 256 + 128]
                        for et in range(ET):
                            nc.tensor.matmul(
                                sl, xtile[:, et, vtt * P : (vtt + 1) * P],
                                wv[:, et, :],
                                start=(et == 0), stop=(et == ET - 1),
                            )
                    for vtt in range(4):
                        tt = xc * 4 + vtt
                        sl = pv[:, vtt * 256 : vtt * 256 + 128]
                        nc.vector.tensor_copy(vt[b][:, tt, 0:64], sl[:, 0:64])
                        nc.vector.tensor_copy(vt[b][:, tt, 65:129],
                                              sl[:, 64:128])

            # ---------- attention ----------
            bg_queue = []

            def drain_bg(n=1):
                for _ in range(n):
                    if not bg_queue:
                        return
                    bg_queue.pop(0)()

            def outproj_chunks(b, tlo, thi):
                """Output projection closures for batch b tokens
                [tlo*128, thi*128)."""
                chunks = []

                def one(br, tt):
                    def go():
                        po = ps_flex.tile([P, D], f32, tag="flex",
                                          name=f"po_{b}_{br}_{tt}")
                        for oc in range(2):
                            nc.tensor.matmul(
                                po[:, oc * 512 : (oc + 1) * 512],
                                oTs[br][b][:, tt * P : (tt + 1) * P],
                                wo[:, oc * 512 : (oc + 1) * 512],
                                start=True,
                                stop=True,
                            )
                        ob = outp.tile([P, D], bf16, tag="ob")
                        nc.vector.tensor_copy(ob[:], po[:])
                        nc.sync.dma_start(
                            out_d[br, b, tt * P : (tt + 1) * P, :], ob[:]
                        )
                    return go

                for tt in range(tlo, thi):
                    for br in range(2):
                        chunks.append(one(br, tt))
                return chunks

            def unit(b, h, qc):
                hp = 64 * h
                vlo, vhi = (0, 65) if h == 0 else (65, 130)
                q0 = qc * QW
                name = f"_{b}_{h}_{qc}"
                vstat = vt[b][:, :, vlo:vhi]

                accm = ps_acc.tile([P, QW], f32, tag="acc",
                                   name=f"accm{name}")
                accp = ps_acc.tile([P, QW], f32, tag="acc",
                                   name=f"accp{name}")
                ems = [None] * KT
                eps = [None] * KT

                def emit_scores(kt):
                    sc = ps_flex.tile([P, QW], f32, tag="flex",
                                      name=f"sc{name}_{kt}")
                    for fh in range(2):
                        nc.tensor.matmul(
                            sc[:, fh * 512 : (fh + 1) * 512],
                            kT[b][hp : hp + 64, kt * P : (kt + 1) * P],
                            qT[b][hp : hp + 64,
                                  q0 + fh * 512 : q0 + (fh + 1) * 512],
                            start=True,
                            stop=True,
                        )
                    em = emp.tile([P, QW], bf16, tag="em", name=f"em{name}_{kt}")
                    nc.scalar.activation(em, sc, Exp, scale=-0.125)
                    ems[kt] = em
                    ep = epp.tile([P, QW], bf16, tag="ep", name=f"ep{name}_{kt}")
                    if kt in ACT_EP_KT:
                        nc.scalar.activation(ep, sc, Exp, scale=0.125)
                    else:
                        # exp(+x) = 1/exp(-x): native DVE reciprocal moves
                        # this half of the exponential work off the scalar
                        # engine; bf16 out is plenty for softmax weights
                        with nc.allow_low_precision(reason="bf16 softmax"):
                            nc.vector.reciprocal(ep, em)
                    eps[kt] = ep

                def emit_pv(acc, e, kt):
                    for fh in range(2):
                        nc.tensor.matmul(
                            acc[0:65, fh * 512 : (fh + 1) * 512],
                            vstat[:, kt, :],
                            e[:, fh * 512 : (fh + 1) * 512],
                            start=(kt == 0),
                            stop=(kt == KT - 1),
                        )

                for kt in range(KT):
                    emit_scores(kt)
                    if kt >= 1:
                        emit_pv(accm, ems[kt - 1], kt - 1)
                        ems[kt - 1] = None
                    if kt >= 2:
                        emit_pv(accp, eps[kt - 2], kt - 2)
                        eps[kt - 2] = None
                    drain_bg(1)
                emit_pv(accm, ems[KT - 1], KT - 1)
                emit_pv(accp, eps[KT - 2], KT - 2)
                emit_pv(accp, eps[KT - 1], KT - 1)

                # ---- normalize: 1/Z = exp(-ln Z), broadcast, mul ----
                lnd = rcpp.tile([P, 2, QW], f32, tag="lnd", name=f"lnd{name}")
                rcp = rcpp.tile([P, 2, QW], bf16, tag="rcp", name=f"rcp{name}")
                for br, acc in ((0, accm), (1, accp)):
                    nc.scalar.activation(lnd[64:65, br, :], acc[64:65, :], Log)
                    nc.scalar.activation(rcp[64:65, br, :], lnd[64:65, br, :],
                                         Exp, scale=-1.0)
                for br, acc in ((0, accm), (1, accp)):
                    bc = ps_flex.tile([P, QW], f32, tag="flex",
                                      name=f"bc{name}_{br}")
                    for fh in range(2):
                        nc.tensor.matmul(
                            bc[hp : hp + 64, fh * 512 : (fh + 1) * 512],
                            ones_sb[64:65, hp : hp + 64],
                            rcp[64:65, br, fh * 512 : (fh + 1) * 512],
                            start=True,
                            stop=True,
                        )
                    oTu = oTup.tile([P, QW], bf16, tag="oTu",
                                    name=f"oTu{name}_{br}")
                    nc.vector.tensor_copy(oTu[0:64, :], acc[0:64, :])
                    if h == 1:
                        oTu2 = oTup.tile([P, QW], bf16, tag="oTu",
                                         name=f"oTu2{name}_{br}")
                        nc.sync.dma_start(oTu2[64:128, :], oTu[0:64, :])
                        oTu = oTu2
                    nc.vector.tensor_mul(
                        oTs[br][b][hp : hp + 64, q0 : q0 + QW],
                        oTu[hp : hp + 64, :],
                        bc[hp : hp + 64, :],
                    )

            for b in range(B):
                for h in range(HPC):
                    for qc in range(QC):
                        unit(b, h, qc)
                        if h == 1:
                            bg_queue.extend(
                                outproj_chunks(b, qc * 8, qc * 8 + 8)
                            )
            drain_bg(len(bg_queue))
    _split_sync_waits(nc)
    return nc


def _get_nc():
    if "nc" not in _compiled:
        _compiled["nc"] = _build()
    return _compiled["nc"]


def _prep_in_maps(x, Wq, bq, Wk, bk, Wv, bv, Wo, bo):
    ET = D // P
    xf = np.ascontiguousarray(x.reshape(B * S, D))
    # x^T tiled: [p, et, token], e = et*128 + p
    xt = np.ascontiguousarray(
        xf.T.reshape(ET, P, B * S).transpose(1, 0, 2)
    ).astype(BF16)
    in_maps = []
    for c in range(NCORES):
        sl = slice(DSL * c, DSL * (c + 1))
        wqt = np.ascontiguousarray(
            Wq[sl].T.reshape(ET, P, DSL).transpose(1, 0, 2)
        ).astype(BF16)
        wkt = np.ascontiguousarray(
            Wk[sl].T.reshape(ET, P, DSL).transpose(1, 0, 2)
        ).astype(BF16)
        wvt = np.ascontiguousarray(
            Wv[sl].T.reshape(ET, P, DSL).transpose(1, 0, 2)
        ).astype(BF16)
        wot = np.ascontiguousarray(Wo[:, sl].T).astype(BF16)
        in_maps.append(
            {
                "xt": xt,
                "wq": wqt,
                "wk": wkt,
                "wv": wvt,
                "wo": wot,
                "bq": np.ascontiguousarray(bq[sl].reshape(P, 1)).astype(np.float32),
                "bk": np.ascontiguousarray(bk[sl].reshape(P, 1)).astype(np.float32),
            }
        )
    return in_maps


def kernel(x, Wq, bq, Wk, bk, Wv, bv, Wo, bo, _trace=False, _tmpdir=None):
    from concourse.bass_utils import run_bass_kernel_spmd

    x, Wq, bq, Wk, bk, Wv, bv, Wo, bo = (
        np.asarray(a, dtype=np.float32)
        for a in (x, Wq, bq, Wk, bk, Wv, bv, Wo, bo)
    )
    nc = _get_nc()
    in_maps = _prep_in_maps(x, Wq, bq, Wk, bk, Wv, bv, Wo, bo)
    res = run_bass_kernel_spmd(
        nc, in_maps, core_ids=list(range(NCORES)), trace=_trace, tmpdir=_tmpdir
    )
    total = np.zeros((2, B, S, D), np.float32)
    for c in range(NCORES):
        total += np.asarray(res.results[c]["out"], dtype=np.float32)
    const_vec = (bv @ Wo.T + bo).astype(np.float32)
    out = total[0] + const_vec
    out_comp = total[1] + const_vec
    if _trace:
        kernel._last_result = res
    return (out, out_comp)


# revision 14
# speedup vs baseline: 1.3860x; 1.3860x over previous
"""Trainium2 Bass kernel for nn_CausalAttention_5815385719336.

Dual-softmax attention: out = softmax(-QK^T/8) V Wo^T (+bias folds),
out_comp = softmax(+QK^T/8) V Wo^T.  B=2, S=2048, D=1024, H=16, DK=64.

Sharding (8 cores): Megatron-style head parallel.  Core c owns heads
(2c, 2c+1) = output dims [128c, 128c+128) of the QKV projections.  Each
core computes its head slice of Q/K/V for both batches, the full [S,S]
attention for its 4 (b, head) units (both softmax branches), and a
partial output projection o_slice @ Wo_slice^T.  The host sums the 8
partial outputs and adds the bias fold (bv @ Wo^T + bo).

v3 engine-balanced pipeline:
  - Prologue runs ALL Q/K/V projections as dense back-to-back matmul
    chains (PE ramps to the full p-state clock; ACT has nothing to do
    that early anyway).
  - exp(-s/8) always comes from ACT (bf16); exp(+s/8) is either a
    second ACT Exp (a tunable subset of kt steps) or a GPSIMD/Pool
    elementwise divide ones/exp(-s) (exp(+x) = 1/exp(-x)), splitting
    the 33M-element exponential work across the scalar engine and the
    otherwise-idle Pool engine.
  - PSUM: flex pool 2x[128,1024] (scores / denom-broadcast / outproj)
    + acc pool 2x[128,1024] (PV+- accumulators with a ones-row
    denominator) = exactly 8 banks.
  - Softmax denominators: ones column in V -> acc row 64; ACT
    Ln/Exp(-x) reciprocal; PE ones-matmul broadcast; DVE copy+mul.
  - Output projection chunks interleave into the next unit's kt loop
    as background PE work; results cast on DVE and DMAd out bf16.
"""

import numpy as np
import ml_dtypes

B, S, D, H, DK = 2, 2048, 1024, 16, 64
NCORES = 8
HPC = H // NCORES          # heads per core = 2
DSL = HPC * DK             # d-slice per core = 128
P = 128
BF16 = ml_dtypes.bfloat16

# kt steps (of 16) whose exp(+s/8) = 1/exp(-s/8) runs on the vector
# engine's (slow but parallel) native reciprocal; the rest are a second
# ACT Exp of the score tile
DVE_EP_KT = frozenset((4, 9, 14))

_compiled = {}


def _install_drain_split():
    """walrus in this container rejects >1 sync wait on the Tile tail
    Drain; split extra waits into standalone wait_ge instructions."""
    import concourse.tile as tile
    from concourse.vector_clock import ScopedClock

    if getattr(tile.TileContext, "_drain_split_installed", False):
        return

    def _drain_and_barrier(self, tick_clock, wait_clock):
        nc = self.nc
        drain_inst = nc.sync.drain()
        wait_clock.add_sem_waits(
            drain_inst.ins, ScopedClock({None: tick_clock.global_clock})
        )
        si = drain_inst.ins.sync_info
        if si is not None and si.on_wait and len(si.on_wait) > 1:
            waits = list(si.on_wait)
            handles = {h.num: h for h in self.sems.allocated().values()}
            si.on_wait = waits[:1]
            for w in waits[1:]:
                assert w.wait_mode == "sem-ge-imm", w.wait_mode
                nc.sync.wait_ge(handles[w.id], w.wait_value)
        nc.all_engine_barrier()
        popped = nc._tile_sem_poison_stack.pop()
        assert popped is self._sem_poison
        nc.clear_and_free_semaphores(list(self.sems.allocated().values()))
        nc.all_engine_barrier()

    tile.TileContext._drain_and_barrier = _drain_and_barrier
    tile.TileContext._drain_split_installed = True


def _split_sync_waits(nc, max_waits=1):
    """walrus in this container has a small per-instruction sync-wait
    capacity.  Hoist excess waits onto standalone EventSemaphore
    instructions inserted just before the owner on the same engine —
    program order within an engine keeps the semantics identical."""
    from concourse import mybir

    n = 0
    for bb in nc.main_func.blocks:
        out = []
        for ins in bb.instructions:
            si = ins.sync_info
            if si is not None and si.on_wait and len(si.on_wait) > max_waits:
                waits = list(si.on_wait)
                for w in waits[:-max_waits]:
                    wi = mybir.InstEventSemaphore(name=f"W-split-{n}", ins=[], outs=[])
                    n += 1
                    wi.engine = ins.engine
                    wi.sync_info = mybir.SyncInfo(on_wait=[w], on_update=[])
                    out.append(wi)
                si.on_wait = waits[-max_waits:]
            out.append(ins)
        if n:
            bb.instructions = out


def _build():
    import concourse.bass as bass
    import concourse.tile as tile
    from concourse import mybir

    _install_drain_split()

    f32 = mybir.dt.float32
    f32r = mybir.dt.float32r
    bf16 = mybir.dt.bfloat16
    Exp = mybir.ActivationFunctionType.Exp
    Log = mybir.ActivationFunctionType.Ln
    NT = B * S                      # 4096 tokens
    ET = D // P                     # 8 e-tiles

    nc = bass.Bass()
    xt_d = nc.declare_dram_parameter("xt", [P, ET, NT], bf16, isOutput=False)
    wq_d = nc.declare_dram_parameter("wq", [P, ET, DSL], bf16, isOutput=False)
    wk_d = nc.declare_dram_parameter("wk", [P, ET, DSL], bf16, isOutput=False)
    wv_d = nc.declare_dram_parameter("wv", [P, ET, DSL], bf16, isOutput=False)
    wo_d = nc.declare_dram_parameter("wo", [P, D], bf16, isOutput=False)
    bq_d = nc.declare_dram_parameter("bq", [P, 1], f32, isOutput=False)
    bk_d = nc.declare_dram_parameter("bk", [P, 1], f32, isOutput=False)
    out_d = nc.declare_dram_parameter("out", [2, B, S, D], bf16, isOutput=True)

    KT = S // P                     # 16 k-tiles per batch
    TT = S // P                     # 16 token-tiles per batch
    QC = 2                          # q chunks per batch
    QW = S // QC                    # 1024
    XC = S // 512                   # 4 x-chunks (512 tokens) per batch

    with tile.TileContext(nc) as tc:
        with (
            tc.tile_pool(name="singles", bufs=1) as singles,
            tc.tile_pool(name="xst", bufs=3) as xst,
            tc.tile_pool(name="emp", bufs=4) as emp,
            tc.tile_pool(name="epp", bufs=4) as epp,
            tc.tile_pool(name="rcpp", bufs=2) as rcpp,
            tc.tile_pool(name="oTup", bufs=4) as oTup,
            tc.tile_pool(name="outp", bufs=3) as outp,
            # 8 PSUM banks: ps_flex 2x[128,1024] = 4 (scores, denom
            # broadcast, projections, outproj), ps_acc 2x = 4 (the two
            # PV accumulators of the active unit).
            tc.tile_pool(name="ps_flex", bufs=2, space="PSUM") as ps_flex,
            tc.tile_pool(name="ps_acc", bufs=2, space="PSUM") as ps_acc,
        ):
            wq = singles.tile([P, ET, DSL], bf16)
            nc.sync.dma_start(wq[:], wq_d[:])
            wk = singles.tile([P, ET, DSL], bf16)
            nc.sync.dma_start(wk[:], wk_d[:])
            wv = singles.tile([P, ET, DSL], bf16)
            nc.sync.dma_start(wv[:], wv_d[:])
            wo = singles.tile([P, D], bf16)
            nc.sync.dma_start(wo[:], wo_d[:])
            bq = singles.tile([P, 1], f32)
            nc.sync.dma_start(bq[:], bq_d[:])
            bk = singles.tile([P, 1], f32)
            nc.sync.dma_start(bk[:], bk_d[:])
            ones_sb = singles.tile([P, P], bf16)
            nc.vector.memset(ones_sb[:], 1.0)
            ones_full = singles.tile([P, QW], bf16)
            nc.vector.memset(ones_full[:], 1.0)

            # persistent per-batch projection outputs
            qT = [singles.tile([P, S], bf16, name=f"qT{b}") for b in range(B)]
            kT = [singles.tile([P, S], bf16, name=f"kT{b}") for b in range(B)]
            vt = [singles.tile([P, TT, 130], bf16, name=f"vt{b}")
                  for b in range(B)]
            for b in range(B):
                nc.vector.memset(vt[b][:, :, 64], 1.0)
                nc.vector.memset(vt[b][:, :, 129], 1.0)
            # oTs[br][b]: normalized o^T slices, bf16
            oTs = [[singles.tile([P, S], bf16, name=f"oTs{br}_{b}")
                    for b in range(B)] for br in range(2)]

            # ---------- prologue: all projections, dense PE chains ----------
            for b in range(B):
                t0 = b * S
                for xc in range(XC):
                    xtile = xst.tile([P, ET, 512], bf16, tag="xt",
                                     name=f"xt_{b}_{xc}")
                    nc.sync.dma_start(
                        xtile[:],
                        xt_d[:, :, t0 + xc * 512 : t0 + (xc + 1) * 512],
                    )
                    # Q and K chains share one flex tile (two 512 halves)
                    pqk = ps_flex.tile([P, 1024], f32, tag="flex",
                                       name=f"pqk_{b}_{xc}")
                    for et in range(ET):
                        nc.tensor.matmul(
                            pqk[:, 0:512], wq[:, et, :], xtile[:, et, :],
                            start=(et == 0), stop=(et == ET - 1),
                        )
                    for et in range(ET):
                        nc.tensor.matmul(
                            pqk[:, 512:1024], wk[:, et, :], xtile[:, et, :],
                            start=(et == 0), stop=(et == ET - 1),
                        )
                    nc.vector.tensor_scalar_add(
                        qT[b][:, xc * 512 : (xc + 1) * 512], pqk[:, 0:512], bq
                    )
                    nc.vector.tensor_scalar_add(
                        kT[b][:, xc * 512 : (xc + 1) * 512], pqk[:, 512:1024],
                        bk,
                    )
                    # V chains: 4 token-tiles per x chunk, tokens on the
                    # out partitions (stationary = x slice)
                    pv = ps_acc.tile([P, 1024], f32, tag="acc",
                                     name=f"pv_{b}_{xc}")
                    for vtt in range(4):
                        tt = xc * 4 + vtt
                        sl = pv[:, vtt * 256 : vtt * 256 + 128]
                        for et in range(ET):
                            nc.tensor.matmul(
                                sl, xtile[:, et, vtt * P : (vtt + 1) * P],
                                wv[:, et, :],
                                start=(et == 0), stop=(et == ET - 1),
                            )
                    for vtt in range(4):
                        tt = xc * 4 + vtt
                        sl = pv[:, vtt * 256 : vtt * 256 + 128]
                        nc.vector.tensor_copy(vt[b][:, tt, 0:64], sl[:, 0:64])
                        nc.vector.tensor_copy(vt[b][:, tt, 65:129],
                                              sl[:, 64:128])

            # ---------- attention ----------
            bg_queue = []

            def drain_bg(n=1):
                for _ in range(n):
                    if not bg_queue:
                        return
                    bg_queue.pop(0)()

            def outproj_chunks(b, tlo, thi):
                """Output projection closures for batch b tokens
                [tlo*128, thi*128)."""
                chunks = []

                def one(br, tt):
                    def go():
                        po = ps_flex.tile([P, D], f32, tag="flex",
                                          name=f"po_{b}_{br}_{tt}")
                        for oc in range(2):
                            nc.tensor.matmul(
                                po[:, oc * 512 : (oc + 1) * 512],
                                oTs[br][b][:, tt * P : (tt + 1) * P],
                                wo[:, oc * 512 : (oc + 1) * 512],
                                start=True,
                                stop=True,
                            )
                        ob = outp.tile([P, D], bf16, tag="ob")
                        nc.vector.tensor_copy(ob[:], po[:])
                        nc.sync.dma_start(
                            out_d[br, b, tt * P : (tt + 1) * P, :], ob[:]
                        )
                    return go

                for tt in range(tlo, thi):
                    for br in range(2):
                        chunks.append(one(br, tt))
                return chunks

            def unit(b, h, qc):
                hp = 64 * h
                vlo, vhi = (0, 65) if h == 0 else (65, 130)
                q0 = qc * QW
                name = f"_{b}_{h}_{qc}"
                vstat = vt[b][:, :, vlo:vhi]

                accm = ps_acc.tile([P, QW], f32, tag="acc",
                                   name=f"accm{name}")
                accp = ps_acc.tile([P, QW], f32, tag="acc",
                                   name=f"accp{name}")
                ems = [None] * KT
                eps = [None] * KT

                def emit_scores(kt):
                    sc = ps_flex.tile([P, QW], f32, tag="flex",
                                      name=f"sc{name}_{kt}")
                    for fh in range(2):
                        nc.tensor.matmul(
                            sc[:, fh * 512 : (fh + 1) * 512],
                            kT[b][hp : hp + 64, kt * P : (kt + 1) * P],
                            qT[b][hp : hp + 64,
                                  q0 + fh * 512 : q0 + (fh + 1) * 512],
                            start=True,
                            stop=True,
                        )
                    em = emp.tile([P, QW], bf16, tag="em", name=f"em{name}_{kt}")
                    nc.scalar.activation(em, sc, Exp, scale=-0.125)
                    ems[kt] = em
                    ep = epp.tile([P, QW], bf16, tag="ep", name=f"ep{name}_{kt}")
                    if kt in DVE_EP_KT:
                        # exp(+x) = 1/exp(-x): native DVE reciprocal moves
                        # part of the exponential work off the scalar
                        # engine; bf16 out is plenty for softmax weights
                        with nc.allow_low_precision(reason="bf16 softmax"):
                            nc.vector.reciprocal(ep, em)
                    else:
                        nc.scalar.activation(ep, sc, Exp, scale=0.125)
                    eps[kt] = ep

                def emit_pv(acc, e, kt):
                    for fh in range(2):
                        nc.tensor.matmul(
                            acc[0:65, fh * 512 : (fh + 1) * 512],
                            vstat[:, kt, :],
                            e[:, fh * 512 : (fh + 1) * 512],
                            start=(kt == 0),
                            stop=(kt == KT - 1),
                        )

                for kt in range(KT):
                    emit_scores(kt)
                    if kt >= 1:
                        emit_pv(accm, ems[kt - 1], kt - 1)
                        ems[kt - 1] = None
                    if kt >= 2:
                        emit_pv(accp, eps[kt - 2], kt - 2)
                        eps[kt - 2] = None
                    drain_bg(1)
                emit_pv(accm, ems[KT - 1], KT - 1)
                emit_pv(accp, eps[KT - 2], KT - 2)
                emit_pv(accp, eps[KT - 1], KT - 1)

                # ---- normalize: 1/Z = exp(-ln Z), broadcast, mul ----
                lnd = rcpp.tile([P, 2, QW], f32, tag="lnd", name=f"lnd{name}")
                rcp = rcpp.tile([P, 2, QW], bf16, tag="rcp", name=f"rcp{name}")
                for br, acc in ((0, accm), (1, accp)):
                    nc.scalar.activation(lnd[64:65, br, :], acc[64:65, :], Log)
                    nc.scalar.activation(rcp[64:65, br, :], lnd[64:65, br, :],
                                         Exp, scale=-1.0)
                for br, acc in ((0, accm), (1, accp)):
                    bc = ps_flex.tile([P, QW], f32, tag="flex",
                                      name=f"bc{name}_{br}")
                    for fh in range(2):
                        nc.tensor.matmul(
                            bc[0:64, fh * 512 : (fh + 1) * 512],
                            ones_sb[64:65, 0:64],
                            rcp[64:65, br, fh * 512 : (fh + 1) * 512],
                            start=True,
                            stop=True,
                        )
                    oTu = oTup.tile([P, QW], bf16, tag="oTu",
                                    name=f"oTu{name}_{br}")
                    nc.vector.tensor_copy(oTu[0:64, :], acc[0:64, :])
                    if h == 0:
                        nc.vector.tensor_mul(
                            oTs[br][b][0:64, q0 : q0 + QW],
                            oTu[0:64, :],
                            bc[0:64, :],
                        )
                    else:
                        oTm = oTup.tile([P, QW], bf16, tag="oTu",
                                        name=f"oTm{name}_{br}")
                        nc.vector.tensor_mul(
                            oTm[0:64, :], oTu[0:64, :], bc[0:64, :]
                        )
                        nc.sync.dma_start(
                            oTs[br][b][64:128, q0 : q0 + QW], oTm[0:64, :]
                        )

            for b in range(B):
                for h in range(HPC):
                    for qc in range(QC):
                        unit(b, h, qc)
                        if h == 1:
                            bg_queue.extend(
                                outproj_chunks(b, qc * 8, qc * 8 + 8)
                            )
            drain_bg(len(bg_queue))
    _split_sync_waits(nc)
    return nc


def _get_nc():
    if "nc" not in _compiled:
        _compiled["nc"] = _build()
    return _compiled["nc"]


def _prep_in_maps(x, Wq, bq, Wk, bk, Wv, bv, Wo, bo):
    ET = D // P
    xf = np.ascontiguousarray(x.reshape(B * S, D))
    # x^T tiled: [p, et, token], e = et*128 + p
    xt = np.ascontiguousarray(
        xf.T.reshape(ET, P, B * S).transpose(1, 0, 2)
    ).astype(BF16)
    in_maps = []
    for c in range(NCORES):
        sl = slice(DSL * c, DSL * (c + 1))
        wqt = np.ascontiguousarray(
            Wq[sl].T.reshape(ET, P, DSL).transpose(1, 0, 2)
        ).astype(BF16)
        wkt = np.ascontiguousarray(
            Wk[sl].T.reshape(ET, P, DSL).transpose(1, 0, 2)
        ).astype(BF16)
        wvt = np.ascontiguousarray(
            Wv[sl].T.reshape(ET, P, DSL).transpose(1, 0, 2)
        ).astype(BF16)
        wot = np.ascontiguousarray(Wo[:, sl].T).astype(BF16)
        in_maps.append(
            {
                "xt": xt,
                "wq": wqt,
                "wk": wkt,
                "wv": wvt,
                "wo": wot,
                "bq": np.ascontiguousarray(bq[sl].reshape(P, 1)).astype(np.float32),
                "bk": np.ascontiguousarray(bk[sl].reshape(P, 1)).astype(np.float32),
            }
        )
    return in_maps


def kernel(x, Wq, bq, Wk, bk, Wv, bv, Wo, bo, _trace=False, _tmpdir=None):
    from concourse.bass_utils import run_bass_kernel_spmd

    x, Wq, bq, Wk, bk, Wv, bv, Wo, bo = (
        np.asarray(a, dtype=np.float32)
        for a in (x, Wq, bq, Wk, bk, Wv, bv, Wo, bo)
    )
    nc = _get_nc()
    in_maps = _prep_in_maps(x, Wq, bq, Wk, bk, Wv, bv, Wo, bo)
    res = run_bass_kernel_spmd(
        nc, in_maps, core_ids=list(range(NCORES)), trace=_trace, tmpdir=_tmpdir
    )
    total = np.zeros((2, B, S, D), np.float32)
    for c in range(NCORES):
        total += np.asarray(res.results[c]["out"], dtype=np.float32)
    const_vec = (bv @ Wo.T + bo).astype(np.float32)
    out = total[0] + const_vec
    out_comp = total[1] + const_vec
    if _trace:
        kernel._last_result = res
    return (out, out_comp)


# revision 18
# speedup vs baseline: 1.3862x; 1.0001x over previous
"""Trainium2 Bass kernel for nn_CausalAttention_5815385719336.

Dual-softmax attention: out = softmax(-QK^T/8) V Wo^T (+bias folds),
out_comp = softmax(+QK^T/8) V Wo^T.  B=2, S=2048, D=1024, H=16, DK=64.

Sharding (8 cores): Megatron-style head parallel.  Core c owns heads
(2c, 2c+1) = output dims [128c, 128c+128) of the QKV projections.  Each
core computes its head slice of Q/K/V for both batches, the full [S,S]
attention for its 4 (b, head) units (both softmax branches), and a
partial output projection o_slice @ Wo_slice^T.  The host sums the 8
partial outputs and adds the bias fold (bv @ Wo^T + bo).

v3 engine-balanced pipeline:
  - Prologue runs ALL Q/K/V projections as dense back-to-back matmul
    chains (PE ramps to the full p-state clock; ACT has nothing to do
    that early anyway).
  - exp(-s/8) always comes from ACT (bf16); exp(+s/8) is either a
    second ACT Exp (a tunable subset of kt steps) or a GPSIMD/Pool
    elementwise divide ones/exp(-s) (exp(+x) = 1/exp(-x)), splitting
    the 33M-element exponential work across the scalar engine and the
    otherwise-idle Pool engine.
  - PSUM: flex pool 2x[128,1024] (scores / denom-broadcast / outproj)
    + acc pool 2x[128,1024] (PV+- accumulators with a ones-row
    denominator) = exactly 8 banks.
  - Softmax denominators: ones column in V -> acc row 64; ACT
    Ln/Exp(-x) reciprocal; PE ones-matmul broadcast; DVE copy+mul.
  - Output projection chunks interleave into the next unit's kt loop
    as background PE work; results cast on DVE and DMAd out bf16.
"""

import numpy as np
import ml_dtypes

B, S, D, H, DK = 2, 2048, 1024, 16, 64
NCORES = 8
HPC = H // NCORES          # heads per core = 2
DSL = HPC * DK             # d-slice per core = 128
P = 128
BF16 = ml_dtypes.bfloat16

# kt steps (of 16) whose exp(+s/8) = 1/exp(-s/8) runs on the vector
# engine's (slow but parallel) native reciprocal; the rest are a second
# ACT Exp of the score tile.  Spread out (recip latency ~6.5us) and away
# from the unit tail.
DVE_EP_KT = frozenset((2, 7, 12))
LAG_M = 2                  # PV- consumes em(kt - LAG_M)
LAG_P = 4                  # PV+ consumes ep(kt - LAG_P)

_compiled = {}


def _install_drain_split():
    """walrus in this container rejects >1 sync wait on the Tile tail
    Drain; split extra waits into standalone wait_ge instructions."""
    import concourse.tile as tile
    from concourse.vector_clock import ScopedClock

    if getattr(tile.TileContext, "_drain_split_installed", False):
        return

    def _drain_and_barrier(self, tick_clock, wait_clock):
        nc = self.nc
        drain_inst = nc.sync.drain()
        wait_clock.add_sem_waits(
            drain_inst.ins, ScopedClock({None: tick_clock.global_clock})
        )
        si = drain_inst.ins.sync_info
        if si is not None and si.on_wait and len(si.on_wait) > 1:
            waits = list(si.on_wait)
            handles = {h.num: h for h in self.sems.allocated().values()}
            si.on_wait = waits[:1]
            for w in waits[1:]:
                assert w.wait_mode == "sem-ge-imm", w.wait_mode
                nc.sync.wait_ge(handles[w.id], w.wait_value)
        nc.all_engine_barrier()
        popped = nc._tile_sem_poison_stack.pop()
        assert popped is self._sem_poison
        nc.clear_and_free_semaphores(list(self.sems.allocated().values()))
        nc.all_engine_barrier()

    tile.TileContext._drain_and_barrier = _drain_and_barrier
    tile.TileContext._drain_split_installed = True


def _split_sync_waits(nc, max_waits=1):
    """walrus in this container has a small per-instruction sync-wait
    capacity.  Hoist excess waits onto standalone EventSemaphore
    instructions inserted just before the owner on the same engine —
    program order within an engine keeps the semantics identical."""
    from concourse import mybir

    n = 0
    for bb in nc.main_func.blocks:
        out = []
        for ins in bb.instructions:
            si = ins.sync_info
            if si is not None and si.on_wait and len(si.on_wait) > max_waits:
                waits = list(si.on_wait)
                for w in waits[:-max_waits]:
                    wi = mybir.InstEventSemaphore(name=f"W-split-{n}", ins=[], outs=[])
                    n += 1
                    wi.engine = ins.engine
                    wi.sync_info = mybir.SyncInfo(on_wait=[w], on_update=[])
                    out.append(wi)
                si.on_wait = waits[-max_waits:]
            out.append(ins)
        if n:
            bb.instructions = out


def _build():
    import concourse.bass as bass
    import concourse.tile as tile
    from concourse import mybir

    _install_drain_split()

    f32 = mybir.dt.float32
    f32r = mybir.dt.float32r
    bf16 = mybir.dt.bfloat16
    Exp = mybir.ActivationFunctionType.Exp
    Log = mybir.ActivationFunctionType.Ln
    NT = B * S                      # 4096 tokens
    ET = D // P                     # 8 e-tiles

    nc = bass.Bass()
    xt_d = nc.declare_dram_parameter("xt", [P, ET, NT], bf16, isOutput=False)
    wq_d = nc.declare_dram_parameter("wq", [P, ET, DSL], bf16, isOutput=False)
    wk_d = nc.declare_dram_parameter("wk", [P, ET, DSL], bf16, isOutput=False)
    wv_d = nc.declare_dram_parameter("wv", [P, ET, DSL], bf16, isOutput=False)
    wo_d = nc.declare_dram_parameter("wo", [P, D], bf16, isOutput=False)
    bq_d = nc.declare_dram_parameter("bq", [P, 1], f32, isOutput=False)
    bk_d = nc.declare_dram_parameter("bk", [P, 1], f32, isOutput=False)
    out_d = nc.declare_dram_parameter("out", [2, B, S, D], bf16, isOutput=True)

    KT = S // P                     # 16 k-tiles per batch
    TT = S // P                     # 16 token-tiles per batch
    QC = 2                          # q chunks per batch
    QW = S // QC                    # 1024
    XC = S // 512                   # 4 x-chunks (512 tokens) per batch

    with tile.TileContext(nc) as tc:
        with (
            tc.tile_pool(name="singles", bufs=1) as singles,
            tc.tile_pool(name="xst", bufs=3) as xst,
            tc.tile_pool(name="emp", bufs=5) as emp,
            tc.tile_pool(name="epp", bufs=7) as epp,
            tc.tile_pool(name="rcpp", bufs=2) as rcpp,
            tc.tile_pool(name="oTup", bufs=4) as oTup,
            tc.tile_pool(name="outp", bufs=3) as outp,
            # 8 PSUM banks: ps_flex 2x[128,1024] = 4 (scores, denom
            # broadcast, projections, outproj), ps_acc 2x = 4 (the two
            # PV accumulators of the active unit).
            tc.tile_pool(name="ps_flex", bufs=2, space="PSUM") as ps_flex,
            tc.tile_pool(name="ps_acc", bufs=2, space="PSUM") as ps_acc,
        ):
            wq = singles.tile([P, ET, DSL], bf16)
            nc.sync.dma_start(wq[:], wq_d[:])
            wk = singles.tile([P, ET, DSL], bf16)
            nc.sync.dma_start(wk[:], wk_d[:])
            wv = singles.tile([P, ET, DSL], bf16)
            nc.sync.dma_start(wv[:], wv_d[:])
            wo = singles.tile([P, D], bf16)
            nc.sync.dma_start(wo[:], wo_d[:])
            bq = singles.tile([P, 1], f32)
            nc.sync.dma_start(bq[:], bq_d[:])
            bk = singles.tile([P, 1], f32)
            nc.sync.dma_start(bk[:], bk_d[:])
            ones_sb = singles.tile([P, P], bf16)
            nc.vector.memset(ones_sb[:], 1.0)
            ones_full = singles.tile([P, QW], bf16)
            nc.vector.memset(ones_full[:], 1.0)

            # persistent per-batch projection outputs
            qT = [singles.tile([P, S], bf16, name=f"qT{b}") for b in range(B)]
            kT = [singles.tile([P, S], bf16, name=f"kT{b}") for b in range(B)]
            vt = [singles.tile([P, TT, 130], bf16, name=f"vt{b}")
                  for b in range(B)]
            for b in range(B):
                nc.vector.memset(vt[b][:, :, 64], 1.0)
                nc.vector.memset(vt[b][:, :, 129], 1.0)
            # oTs[br][b]: normalized o^T slices, bf16
            oTs = [[singles.tile([P, S], bf16, name=f"oTs{br}_{b}")
                    for b in range(B)] for br in range(2)]

            # ---------- projections ----------
            # batch 0 runs eagerly (dense PE chains ramp the p-state);
            # batch 1 is emitted as closures drained at unit boundaries
            # of batch-0 attention, filling the PE while accumulators and
            # normalize chains turn over.
            def proj_qk(b, xc, xtile, ps_pool, ptag):
                # Q and K chains share one flex tile (two 512 halves)
                pqk = ps_pool.tile([P, 1024], f32, tag=ptag,
                                   name=f"pqk_{b}_{xc}")
                for et in range(ET):
                    nc.tensor.matmul(
                        pqk[:, 0:512], wq[:, et, :], xtile[:, et, :],
                        start=(et == 0), stop=(et == ET - 1),
                    )
                for et in range(ET):
                    nc.tensor.matmul(
                        pqk[:, 512:1024], wk[:, et, :], xtile[:, et, :],
                        start=(et == 0), stop=(et == ET - 1),
                    )
                nc.vector.tensor_scalar_add(
                    qT[b][:, xc * 512 : (xc + 1) * 512], pqk[:, 0:512], bq
                )
                nc.vector.tensor_scalar_add(
                    kT[b][:, xc * 512 : (xc + 1) * 512], pqk[:, 512:1024], bk
                )

            def proj_v(b, xc, xtile, ps_pool, ptag):
                # V chains: 4 token-tiles per x chunk, tokens on the out
                # partitions (stationary = x slice)
                pv = ps_pool.tile([P, 1024], f32, tag=ptag,
                                  name=f"pv_{b}_{xc}")
                for vtt in range(4):
                    sl = pv[:, vtt * 256 : vtt * 256 + 128]
                    for et in range(ET):
                        nc.tensor.matmul(
                            sl, xtile[:, et, vtt * P : (vtt + 1) * P],
                            wv[:, et, :],
                            start=(et == 0), stop=(et == ET - 1),
                        )
                for vtt in range(4):
                    tt = xc * 4 + vtt
                    sl = pv[:, vtt * 256 : vtt * 256 + 128]
                    nc.vector.tensor_copy(vt[b][:, tt, 0:64], sl[:, 0:64])
                    nc.vector.tensor_copy(vt[b][:, tt, 65:129], sl[:, 64:128])

            def load_x(b, xc):
                xtile = xst.tile([P, ET, 512], bf16, tag="xt",
                                 name=f"xt_{b}_{xc}")
                nc.sync.dma_start(
                    xtile[:],
                    xt_d[:, :, b * S + xc * 512 : b * S + (xc + 1) * 512],
                )
                return xtile

            for xc in range(XC):
                xtile = load_x(0, xc)
                proj_qk(0, xc, xtile, ps_flex, "flex")
                proj_v(0, xc, xtile, ps_acc, "acc")

            b1_proj = []
            _b1_cell = {}

            def _b1_chunk(xc, which):
                def go():
                    if xc not in _b1_cell:
                        _b1_cell[xc] = load_x(1, xc)
                    if which == "qk":
                        proj_qk(1, xc, _b1_cell[xc], ps_flex, "flex")
                    else:
                        proj_v(1, xc, _b1_cell[xc], ps_flex, "flex")
                return go

            for xc in range(XC):
                b1_proj.append(_b1_chunk(xc, "qk"))
                b1_proj.append(_b1_chunk(xc, "v"))

            # ---------- attention ----------
            # bg_step: light chunks (outproj) drained one per kt step;
            # bg_bound: fat chunks (batch-1 projections) drained only at
            # unit boundaries where the PE would otherwise stall on the
            # accumulator turnover.
            bg_step = []
            bg_bound = []

            def drain_step(n=1):
                for _ in range(n):
                    if not bg_step:
                        return
                    bg_step.pop(0)()

            def drain_bound(n=1):
                for _ in range(n):
                    if bg_bound:
                        bg_bound.pop(0)()
                    elif bg_step:
                        bg_step.pop(0)()
                    else:
                        return

            def outproj_chunks(b, tlo, thi):
                """Output projection closures for batch b tokens
                [tlo*128, thi*128)."""
                chunks = []

                def one(br, tt):
                    def go():
                        po = ps_flex.tile([P, D], f32, tag="flex",
                                          name=f"po_{b}_{br}_{tt}")
                        for oc in range(2):
                            nc.tensor.matmul(
                                po[:, oc * 512 : (oc + 1) * 512],
                                oTs[br][b][:, tt * P : (tt + 1) * P],
                                wo[:, oc * 512 : (oc + 1) * 512],
                                start=True,
                                stop=True,
                            )
                        ob = outp.tile([P, D], bf16, tag="ob")
                        nc.vector.tensor_copy(ob[:], po[:])
                        nc.sync.dma_start(
                            out_d[br, b, tt * P : (tt + 1) * P, :], ob[:]
                        )
                    return go

                for tt in range(tlo, thi):
                    for br in range(2):
                        chunks.append(one(br, tt))
                return chunks

            def unit(b, h, qc):
                hp = 64 * h
                vlo, vhi = (0, 65) if h == 0 else (65, 130)
                q0 = qc * QW
                name = f"_{b}_{h}_{qc}"
                vstat = vt[b][:, :, vlo:vhi]

                accm = ps_acc.tile([P, QW], f32, tag="acc",
                                   name=f"accm{name}")
                accp = ps_acc.tile([P, QW], f32, tag="acc",
                                   name=f"accp{name}")
                ems = [None] * KT
                eps = [None] * KT

                def emit_scores(kt):
                    sc = ps_flex.tile([P, QW], f32, tag="flex",
                                      name=f"sc{name}_{kt}")
                    for fh in range(2):
                        nc.tensor.matmul(
                            sc[:, fh * 512 : (fh + 1) * 512],
                            kT[b][hp : hp + 64, kt * P : (kt + 1) * P],
                            qT[b][hp : hp + 64,
                                  q0 + fh * 512 : q0 + (fh + 1) * 512],
                            start=True,
                            stop=True,
                        )
                    em = emp.tile([P, QW], bf16, tag="em", name=f"em{name}_{kt}")
                    nc.scalar.activation(em, sc, Exp, scale=-0.125)
                    ems[kt] = em
                    ep = epp.tile([P, QW], bf16, tag="ep", name=f"ep{name}_{kt}")
                    if kt in DVE_EP_KT:
                        # exp(+x) = 1/exp(-x): native DVE reciprocal moves
                        # part of the exponential work off the scalar
                        # engine; bf16 out is plenty for softmax weights
                        with nc.allow_low_precision(reason="bf16 softmax"):
                            nc.vector.reciprocal(ep, em)
                    else:
                        nc.scalar.activation(ep, sc, Exp, scale=0.125)
                    eps[kt] = ep

                def emit_pv(acc, e, kt):
                    for fh in range(2):
                        nc.tensor.matmul(
                            acc[0:65, fh * 512 : (fh + 1) * 512],
                            vstat[:, kt, :],
                            e[:, fh * 512 : (fh + 1) * 512],
                            start=(kt == 0),
                            stop=(kt == KT - 1),
                        )

                for kt in range(KT):
                    emit_scores(kt)
                    if kt >= LAG_M:
                        emit_pv(accm, ems[kt - LAG_M], kt - LAG_M)
                        ems[kt - LAG_M] = None
                    if kt >= LAG_P:
                        emit_pv(accp, eps[kt - LAG_P], kt - LAG_P)
                        eps[kt - LAG_P] = None
                    drain_step(1)
                for kt in range(KT - LAG_M, KT):
                    emit_pv(accm, ems[kt], kt)
                for kt in range(KT - LAG_P, KT):
                    emit_pv(accp, eps[kt], kt)

                # ---- drain the accumulators fast: one bf16 copy each
                # (rows 0:65 = unnormalized oT + the denominator row).
                # Everything downstream (Ln/Exp/broadcast/mul) reads the
                # copy, so the next unit's PV can reuse the PSUM banks
                # after ~2us instead of waiting on the normalize chain.
                oTu = {}
                for br, acc in ((0, accm), (1, accp)):
                    t = oTup.tile([P, QW], bf16, tag="oTu",
                                  name=f"oTu{name}_{br}")
                    nc.vector.tensor_copy(t[0:65, :], acc[0:65, :])
                    oTu[br] = t
                drain_bound(1)

                # ---- normalize: 1/Z = exp(-ln Z), broadcast, mul ----
                lnd = rcpp.tile([P, 2, QW], f32, tag="lnd", name=f"lnd{name}")
                rcp = rcpp.tile([P, 2, QW], bf16, tag="rcp", name=f"rcp{name}")
                for br in range(2):
                    nc.scalar.activation(lnd[64:65, br, :], oTu[br][64:65, :],
                                         Log)
                    nc.scalar.activation(rcp[64:65, br, :], lnd[64:65, br, :],
                                         Exp, scale=-1.0)
                drain_bound(1)
                for br in range(2):
                    bc = ps_flex.tile([P, QW], f32, tag="flex",
                                      name=f"bc{name}_{br}")
                    for fh in range(2):
                        nc.tensor.matmul(
                            bc[0:64, fh * 512 : (fh + 1) * 512],
                            ones_sb[64:65, 0:64],
                            rcp[64:65, br, fh * 512 : (fh + 1) * 512],
                            start=True,
                            stop=True,
                        )
                    if h == 0:
                        nc.vector.tensor_mul(
                            oTs[br][b][0:64, q0 : q0 + QW],
                            oTu[br][0:64, :],
                            bc[0:64, :],
                        )
                    else:
                        oTm = oTup.tile([P, QW], bf16, tag="oTu",
                                        name=f"oTm{name}_{br}")
                        nc.vector.tensor_mul(
                            oTm[0:64, :], oTu[br][0:64, :], bc[0:64, :]
                        )
                        nc.sync.dma_start(
                            oTs[br][b][64:128, q0 : q0 + QW], oTm[0:64, :]
                        )
                drain_bound(1)

            bg_bound.extend(b1_proj)
            for b in range(B):
                if b == 1:
                    drain_bound(len(bg_bound))
                for qc in range(QC):
                    for h in range(HPC):
                        unit(b, h, qc)
                        if h == 1:
                            bg_step.extend(
                                outproj_chunks(b, qc * 8, qc * 8 + 8)
                            )
            drain_bound(len(bg_bound) + len(bg_step))
    _split_sync_waits(nc)
    return nc


def _get_nc():
    if "nc" not in _compiled:
        _compiled["nc"] = _build()
    return _compiled["nc"]


def _prep_in_maps(x, Wq, bq, Wk, bk, Wv, bv, Wo, bo):
    ET = D // P
    xf = np.ascontiguousarray(x.reshape(B * S, D))
    # x^T tiled: [p, et, token], e = et*128 + p
    xt = np.ascontiguousarray(
        xf.T.reshape(ET, P, B * S).transpose(1, 0, 2)
    ).astype(BF16)
    in_maps = []
    for c in range(NCORES):
        sl = slice(DSL * c, DSL * (c + 1))
        wqt = np.ascontiguousarray(
            Wq[sl].T.reshape(ET, P, DSL).transpose(1, 0, 2)
        ).astype(BF16)
        wkt = np.ascontiguousarray(
            Wk[sl].T.reshape(ET, P, DSL).transpose(1, 0, 2)
        ).astype(BF16)
        wvt = np.ascontiguousarray(
            Wv[sl].T.reshape(ET, P, DSL).transpose(1, 0, 2)
        ).astype(BF16)
        wot = np.ascontiguousarray(Wo[:, sl].T).astype(BF16)
        in_maps.append(
            {
                "xt": xt,
                "wq": wqt,
                "wk": wkt,
                "wv": wvt,
                "wo": wot,
                "bq": np.ascontiguousarray(bq[sl].reshape(P, 1)).astype(np.float32),
                "bk": np.ascontiguousarray(bk[sl].reshape(P, 1)).astype(np.float32),
            }
        )
    return in_maps


def kernel(x, Wq, bq, Wk, bk, Wv, bv, Wo, bo, _trace=False, _tmpdir=None):
    from concourse.bass_utils import run_bass_kernel_spmd

    x, Wq, bq, Wk, bk, Wv, bv, Wo, bo = (
        np.asarray(a, dtype=np.float32)
        for a in (x, Wq, bq, Wk, bk, Wv, bv, Wo, bo)
    )
    nc = _get_nc()
    in_maps = _prep_in_maps(x, Wq, bq, Wk, bk, Wv, bv, Wo, bo)
    res = run_bass_kernel_spmd(
        nc, in_maps, core_ids=list(range(NCORES)), trace=_trace, tmpdir=_tmpdir
    )
    total = np.zeros((2, B, S, D), np.float32)
    for c in range(NCORES):
        total += np.asarray(res.results[c]["out"], dtype=np.float32)
    const_vec = (bv @ Wo.T + bo).astype(np.float32)
    out = total[0] + const_vec
    out_comp = total[1] + const_vec
    if _trace:
        kernel._last_result = res
    return (out, out_comp)


# revision 22
# speedup vs baseline: 1.9658x; 1.4181x over previous
"""Trainium2 Bass kernel for nn_CausalAttention_5815385719336.

Dual-softmax attention: out = softmax(-QK^T/8) V Wo^T (+bias folds),
out_comp = softmax(+QK^T/8) V Wo^T.  B=2, S=2048, D=1024, H=16, DK=64.

Sharding (8 cores): Megatron-style head parallel.  Core c owns heads
(2c, 2c+1) = output dims [128c, 128c+128) of the QKV projections.  Each
core computes its head slice of Q/K/V for both batches, the full [S,S]
attention for its 4 (b, head) units (both softmax branches), and a
partial output projection o_slice @ Wo_slice^T.  The host sums the 8
partial outputs and adds the bias fold (bv @ Wo^T + bo).

v3 engine-balanced pipeline:
  - Prologue runs ALL Q/K/V projections as dense back-to-back matmul
    chains (PE ramps to the full p-state clock; ACT has nothing to do
    that early anyway).
  - exp(-s/8) always comes from ACT (bf16); exp(+s/8) is either a
    second ACT Exp (a tunable subset of kt steps) or a GPSIMD/Pool
    elementwise divide ones/exp(-s) (exp(+x) = 1/exp(-x)), splitting
    the 33M-element exponential work across the scalar engine and the
    otherwise-idle Pool engine.
  - PSUM: flex pool 2x[128,1024] (scores / denom-broadcast / outproj)
    + acc pool 2x[128,1024] (PV+- accumulators with a ones-row
    denominator) = exactly 8 banks.
  - Softmax denominators: ones column in V -> acc row 64; ACT
    Ln/Exp(-x) reciprocal; PE ones-matmul broadcast; DVE copy+mul.
  - Output projection chunks interleave into the next unit's kt loop
    as background PE work; results cast on DVE and DMAd out bf16.
"""

import numpy as np
import ml_dtypes

B, S, D, H, DK = 2, 2048, 1024, 16, 64
NCORES = 8
HPC = H // NCORES          # heads per core = 2
DSL = HPC * DK             # d-slice per core = 128
P = 128
BF16 = ml_dtypes.bfloat16

# kt steps (of 16) whose exp(+s/8) = 1/exp(-s/8) runs on the vector
# engine's (slow but parallel) native reciprocal; the rest are a second
# ACT Exp of the score tile.  Spread out (recip latency ~6.5us) and away
# from the unit tail.
DVE_EP_KT = frozenset((1, 3, 5, 8, 10, 12, 14))
LAG_M = 2                  # PV- consumes em(kt - LAG_M)
LAG_P = 4                  # PV+ consumes ep(kt - LAG_P)
HEAD = 2                   # steps of the next unit emitted before the
                           # previous unit's trailing PVs / normalize
RECIP_MAGIC = 0x7EF3       # bf16 reciprocal-seed magic: seed = M - bits(x)

_compiled = {}


def _install_drain_split():
    """walrus in this container rejects >1 sync wait on the Tile tail
    Drain; split extra waits into standalone wait_ge instructions."""
    import concourse.tile as tile
    from concourse.vector_clock import ScopedClock

    if getattr(tile.TileContext, "_drain_split_installed", False):
        return

    def _drain_and_barrier(self, tick_clock, wait_clock):
        nc = self.nc
        drain_inst = nc.sync.drain()
        wait_clock.add_sem_waits(
            drain_inst.ins, ScopedClock({None: tick_clock.global_clock})
        )
        si = drain_inst.ins.sync_info
        if si is not None and si.on_wait and len(si.on_wait) > 1:
            waits = list(si.on_wait)
            handles = {h.num: h for h in self.sems.allocated().values()}
            si.on_wait = waits[:1]
            for w in waits[1:]:
                assert w.wait_mode == "sem-ge-imm", w.wait_mode
                nc.sync.wait_ge(handles[w.id], w.wait_value)
        nc.all_engine_barrier()
        popped = nc._tile_sem_poison_stack.pop()
        assert popped is self._sem_poison
        nc.clear_and_free_semaphores(list(self.sems.allocated().values()))
        nc.all_engine_barrier()

    tile.TileContext._drain_and_barrier = _drain_and_barrier
    tile.TileContext._drain_split_installed = True


def _split_sync_waits(nc, max_waits=1):
    """walrus in this container has a small per-instruction sync-wait
    capacity.  Hoist excess waits onto standalone EventSemaphore
    instructions inserted just before the owner on the same engine —
    program order within an engine keeps the semantics identical."""
    from concourse import mybir

    n = 0
    for bb in nc.main_func.blocks:
        out = []
        for ins in bb.instructions:
            si = ins.sync_info
            if si is not None and si.on_wait and len(si.on_wait) > max_waits:
                waits = list(si.on_wait)
                for w in waits[:-max_waits]:
                    wi = mybir.InstEventSemaphore(name=f"W-split-{n}", ins=[], outs=[])
                    n += 1
                    wi.engine = ins.engine
                    wi.sync_info = mybir.SyncInfo(on_wait=[w], on_update=[])
                    out.append(wi)
                si.on_wait = waits[-max_waits:]
            out.append(ins)
        if n:
            bb.instructions = out


def _build():
    import concourse.bass as bass
    import concourse.tile as tile
    from concourse import mybir

    _install_drain_split()

    f32 = mybir.dt.float32
    u16 = mybir.dt.uint16
    bf16 = mybir.dt.bfloat16
    Exp = mybir.ActivationFunctionType.Exp
    Log = mybir.ActivationFunctionType.Ln
    NT = B * S                      # 4096 tokens
    ET = D // P                     # 8 e-tiles

    nc = bass.Bass()
    xt_d = nc.declare_dram_parameter("xt", [P, ET, NT], bf16, isOutput=False)
    wq_d = nc.declare_dram_parameter("wq", [P, ET, DSL], bf16, isOutput=False)
    wk_d = nc.declare_dram_parameter("wk", [P, ET, DSL], bf16, isOutput=False)
    wv_d = nc.declare_dram_parameter("wv", [P, ET, DSL], bf16, isOutput=False)
    wo_d = nc.declare_dram_parameter("wo", [P, D], bf16, isOutput=False)
    bq_d = nc.declare_dram_parameter("bq", [P, 1], f32, isOutput=False)
    bk_d = nc.declare_dram_parameter("bk", [P, 1], f32, isOutput=False)
    out_d = nc.declare_dram_parameter("out", [2, B, S, D], bf16, isOutput=True)

    KT = S // P                     # 16 k-tiles per batch
    TT = S // P                     # 16 token-tiles per batch
    QC = 2                          # q chunks per batch
    QW = S // QC                    # 1024
    XC = S // 512                   # 4 x-chunks (512 tokens) per batch

    with tile.TileContext(nc) as tc:
        with (
            tc.tile_pool(name="singles", bufs=1) as singles,
            tc.tile_pool(name="xst", bufs=3) as xst,
            tc.tile_pool(name="emp", bufs=5) as emp,
            tc.tile_pool(name="epp", bufs=7) as epp,
            tc.tile_pool(name="rcpp", bufs=2) as rcpp,
            tc.tile_pool(name="hkp", bufs=4) as hkp,
            tc.tile_pool(name="oTup", bufs=4) as oTup,
            tc.tile_pool(name="outp", bufs=3) as outp,
            # 8 PSUM banks: ps_flex 2x[128,1024] = 4 (scores, denom
            # broadcast, projections, outproj), ps_acc 2x = 4 (the two
            # PV accumulators of the active unit).
            tc.tile_pool(name="ps_flex", bufs=2, space="PSUM") as ps_flex,
            tc.tile_pool(name="ps_acc", bufs=2, space="PSUM") as ps_acc,
        ):
            wq = singles.tile([P, ET, DSL], bf16)
            nc.sync.dma_start(wq[:], wq_d[:])
            wk = singles.tile([P, ET, DSL], bf16)
            nc.sync.dma_start(wk[:], wk_d[:])
            wv = singles.tile([P, ET, DSL], bf16)
            nc.sync.dma_start(wv[:], wv_d[:])
            wo = singles.tile([P, D], bf16)
            nc.sync.dma_start(wo[:], wo_d[:])
            bq = singles.tile([P, 1], f32)
            nc.sync.dma_start(bq[:], bq_d[:])
            bk = singles.tile([P, 1], f32)
            nc.sync.dma_start(bk[:], bk_d[:])
            ones_sb = singles.tile([P, P], bf16)
            nc.vector.memset(ones_sb[:], 1.0)
            ones_full = singles.tile([P, QW], bf16)
            nc.vector.memset(ones_full[:], 1.0)

            # persistent per-batch projection outputs
            qT = [singles.tile([P, S], bf16, name=f"qT{b}") for b in range(B)]
            kT = [singles.tile([P, S], bf16, name=f"kT{b}") for b in range(B)]
            vt = [singles.tile([P, TT, 130], bf16, name=f"vt{b}")
                  for b in range(B)]
            for b in range(B):
                nc.vector.memset(vt[b][:, :, 64], 1.0)
                nc.vector.memset(vt[b][:, :, 129], 1.0)
            # oTs[br][b]: normalized o^T slices, bf16
            oTs = [[singles.tile([P, S], bf16, name=f"oTs{br}_{b}")
                    for b in range(B)] for br in range(2)]

            # ---------- projections ----------
            # batch 0 runs eagerly (dense PE chains ramp the p-state);
            # batch 1 is emitted as closures drained at unit boundaries
            # of batch-0 attention, filling the PE while accumulators and
            # normalize chains turn over.
            def proj_qk(b, xc, xtile, ps_pool, ptag):
                # Q and K chains share one flex tile (two 512 halves)
                pqk = ps_pool.tile([P, 1024], f32, tag=ptag,
                                   name=f"pqk_{b}_{xc}")
                for et in range(ET):
                    nc.tensor.matmul(
                        pqk[:, 0:512], wq[:, et, :], xtile[:, et, :],
                        start=(et == 0), stop=(et == ET - 1),
                    )
                for et in range(ET):
                    nc.tensor.matmul(
                        pqk[:, 512:1024], wk[:, et, :], xtile[:, et, :],
                        start=(et == 0), stop=(et == ET - 1),
                    )
                nc.vector.tensor_scalar_add(
                    qT[b][:, xc * 512 : (xc + 1) * 512], pqk[:, 0:512], bq
                )
                nc.vector.tensor_scalar_add(
                    kT[b][:, xc * 512 : (xc + 1) * 512], pqk[:, 512:1024], bk
                )

            def proj_v(b, xc, xtile, ps_pool, ptag):
                # V chains: 4 token-tiles per x chunk, tokens on the out
                # partitions (stationary = x slice)
                pv = ps_pool.tile([P, 1024], f32, tag=ptag,
                                  name=f"pv_{b}_{xc}")
                for vtt in range(4):
                    sl = pv[:, vtt * 256 : vtt * 256 + 128]
                    for et in range(ET):
                        nc.tensor.matmul(
                            sl, xtile[:, et, vtt * P : (vtt + 1) * P],
                            wv[:, et, :],
                            start=(et == 0), stop=(et == ET - 1),
                        )
                for vtt in range(4):
                    tt = xc * 4 + vtt
                    sl = pv[:, vtt * 256 : vtt * 256 + 128]
                    nc.vector.tensor_copy(vt[b][:, tt, 0:64], sl[:, 0:64])
                    nc.vector.tensor_copy(vt[b][:, tt, 65:129], sl[:, 64:128])

            def load_x(b, xc):
                xtile = xst.tile([P, ET, 512], bf16, tag="xt",
                                 name=f"xt_{b}_{xc}")
                nc.sync.dma_start(
                    xtile[:],
                    xt_d[:, :, b * S + xc * 512 : b * S + (xc + 1) * 512],
                )
                return xtile

            for xc in range(XC):
                xtile = load_x(0, xc)
                proj_qk(0, xc, xtile, ps_flex, "flex")
                proj_v(0, xc, xtile, ps_acc, "acc")

            b1_proj = []
            _b1_cell = {}

            def _b1_chunk(xc, which):
                def go():
                    if xc not in _b1_cell:
                        _b1_cell[xc] = load_x(1, xc)
                    if which == "qk":
                        proj_qk(1, xc, _b1_cell[xc], ps_flex, "flex")
                    else:
                        proj_v(1, xc, _b1_cell[xc], ps_flex, "flex")
                return go

            for xc in range(XC):
                b1_proj.append(_b1_chunk(xc, "qk"))
                b1_proj.append(_b1_chunk(xc, "v"))

            # ---------- attention ----------
            # bg_step: light chunks (outproj) drained one per kt step;
            # bg_bound: fat chunks (batch-1 projections) drained only at
            # unit boundaries where the PE would otherwise stall on the
            # accumulator turnover.
            bg_step = []
            bg_bound = []

            def drain_step(n=1):
                for _ in range(n):
                    if not bg_step:
                        return
                    bg_step.pop(0)()

            def drain_bound(n=1):
                for _ in range(n):
                    if bg_bound:
                        bg_bound.pop(0)()
                    elif bg_step:
                        bg_step.pop(0)()
                    else:
                        return

            def outproj_chunks(b, tlo, thi):
                """Output projection closures for batch b tokens
                [tlo*128, thi*128)."""
                chunks = []

                def one(br, tt):
                    def go():
                        po = ps_flex.tile([P, D], f32, tag="flex",
                                          name=f"po_{b}_{br}_{tt}")
                        for oc in range(2):
                            nc.tensor.matmul(
                                po[:, oc * 512 : (oc + 1) * 512],
                                oTs[br][b][:, tt * P : (tt + 1) * P],
                                wo[:, oc * 512 : (oc + 1) * 512],
                                start=True,
                                stop=True,
                            )
                        ob = outp.tile([P, D], bf16, tag="ob")
                        nc.vector.tensor_copy(ob[:], po[:])
                        nc.sync.dma_start(
                            out_d[br, b, tt * P : (tt + 1) * P, :], ob[:]
                        )
                    return go

                for tt in range(tlo, thi):
                    for br in range(2):
                        chunks.append(one(br, tt))
                return chunks

            def make_unit(b, h, qc):
                hp = 64 * h
                vlo, vhi = (0, 65) if h == 0 else (65, 130)
                q0 = qc * QW
                name = f"_{b}_{h}_{qc}"
                vstat = vt[b][:, :, vlo:vhi]

                accm = ps_acc.tile([P, QW], f32, tag="acc",
                                   name=f"accm{name}")
                accp = ps_acc.tile([P, QW], f32, tag="acc",
                                   name=f"accp{name}")
                ems = [None] * KT
                eps = [None] * KT

                def emit_scores(kt):
                    sc = ps_flex.tile([P, QW], f32, tag="flex",
                                      name=f"sc{name}_{kt}")
                    for fh in range(2):
                        nc.tensor.matmul(
                            sc[:, fh * 512 : (fh + 1) * 512],
                            kT[b][hp : hp + 64, kt * P : (kt + 1) * P],
                            qT[b][hp : hp + 64,
                                  q0 + fh * 512 : q0 + (fh + 1) * 512],
                            start=True,
                            stop=True,
                        )
                    em = emp.tile([P, QW], bf16, tag="em", name=f"em{name}_{kt}")
                    nc.scalar.activation(em, sc, Exp, scale=-0.125)
                    ems[kt] = em
                    ep = epp.tile([P, QW], bf16, tag="ep", name=f"ep{name}_{kt}")
                    if kt in DVE_EP_KT:
                        # exp(+x) = 1/exp(-x) on the vector engine: bf16
                        # bit-hack seed (one u16 xor/add pass) plus a fused
                        # Newton step (two scalar_tensor_tensor passes);
                        # max rel err ~0.7%, all three ops at the 16-bit
                        # DVE rate.  Moves ~half the exponential work off
                        # the saturated scalar engine.
                        r0 = hkp.tile([P, QW], bf16, tag="hk",
                                      name=f"r0{name}_{kt}")
                        nt = hkp.tile([P, QW], bf16, tag="hk",
                                      name=f"nt{name}_{kt}")
                        nc.vector.tensor_scalar(
                            out=r0.bitcast(u16),
                            in0=em.bitcast(u16),
                            scalar1=RECIP_MAGIC,
                            scalar2=-1,
                            op0=mybir.AluOpType.subtract,
                            op1=mybir.AluOpType.mult,
                        )
                        nc.vector.scalar_tensor_tensor(
                            out=nt, in0=em, scalar=-1.0, in1=r0,
                            op0=mybir.AluOpType.mult,
                            op1=mybir.AluOpType.mult,
                        )
                        nc.vector.scalar_tensor_tensor(
                            out=ep, in0=nt, scalar=2.0, in1=r0,
                            op0=mybir.AluOpType.add,
                            op1=mybir.AluOpType.mult,
                        )
                    else:
                        nc.scalar.activation(ep, sc, Exp, scale=0.125)
                    eps[kt] = ep

                def emit_pv(acc, e, kt):
                    for fh in range(2):
                        nc.tensor.matmul(
                            acc[0:65, fh * 512 : (fh + 1) * 512],
                            vstat[:, kt, :],
                            e[:, fh * 512 : (fh + 1) * 512],
                            start=(kt == 0),
                            stop=(kt == KT - 1),
                        )

                def head():
                    for kt in range(HEAD):
                        emit_scores(kt)

                def body():
                    for kt in range(HEAD, KT):
                        emit_scores(kt)
                        if kt >= LAG_M:
                            emit_pv(accm, ems[kt - LAG_M], kt - LAG_M)
                            ems[kt - LAG_M] = None
                        if kt >= LAG_P:
                            emit_pv(accp, eps[kt - LAG_P], kt - LAG_P)
                            eps[kt - LAG_P] = None
                        drain_step(1)

                def tail():
                    for kt in range(KT - LAG_M, KT):
                        emit_pv(accm, ems[kt], kt)
                    for kt in range(KT - LAG_P, KT):
                        emit_pv(accp, eps[kt], kt)

                    # ---- drain the accumulators fast: one bf16 copy
                    # each (rows 0:65 = unnormalized oT + denominator).
                    # Everything downstream (Ln/Exp/broadcast/mul) reads
                    # the copy, so the next unit's PV can reuse the PSUM
                    # banks after ~2us instead of waiting on the
                    # normalize chain.
                    oTu = {}
                    for br, acc in ((0, accm), (1, accp)):
                        t = oTup.tile([P, QW], bf16, tag="oTu",
                                      name=f"oTu{name}_{br}")
                        nc.vector.tensor_copy(t[0:65, :], acc[0:65, :])
                        oTu[br] = t
                    drain_bound(1)

                    # ---- normalize: 1/Z = exp(-ln Z), broadcast, mul ----
                    lnd = rcpp.tile([P, 2, QW], f32, tag="lnd",
                                    name=f"lnd{name}")
                    rcp = rcpp.tile([P, 2, QW], bf16, tag="rcp",
                                    name=f"rcp{name}")
                    for br in range(2):
                        nc.scalar.activation(lnd[64:65, br, :],
                                             oTu[br][64:65, :], Log)
                        nc.scalar.activation(rcp[64:65, br, :],
                                             lnd[64:65, br, :], Exp,
                                             scale=-1.0)
                    drain_bound(1)
                    for br in range(2):
                        bc = ps_flex.tile([P, QW], f32, tag="flex",
                                          name=f"bc{name}_{br}")
                        for fh in range(2):
                            nc.tensor.matmul(
                                bc[0:64, fh * 512 : (fh + 1) * 512],
                                ones_sb[64:65, 0:64],
                                rcp[64:65, br, fh * 512 : (fh + 1) * 512],
                                start=True,
                                stop=True,
                            )
                        if h == 0:
                            nc.vector.tensor_mul(
                                oTs[br][b][0:64, q0 : q0 + QW],
                                oTu[br][0:64, :],
                                bc[0:64, :],
                            )
                        else:
                            oTm = oTup.tile([P, QW], bf16, tag="oTu",
                                            name=f"oTm{name}_{br}")
                            nc.vector.tensor_mul(
                                oTm[0:64, :], oTu[br][0:64, :], bc[0:64, :]
                            )
                            nc.sync.dma_start(
                                oTs[br][b][64:128, q0 : q0 + QW],
                                oTm[0:64, :],
                            )
                    drain_bound(1)

                return head, body, tail

            bg_bound.extend(b1_proj)
            plan = [(b, qc, h)
                    for b in range(B) for qc in range(QC) for h in range(HPC)]
            prev_tail = None
            prev_u = None
            for u in plan:
                b, qc, h = u
                uh, ub, ut = make_unit(b, h, qc)
                uh()                     # next unit's first scores...
                if prev_tail is not None:
                    prev_tail()          # ...before the previous trailing
                    if prev_u[2] == 1:
                        bg_step.extend(
                            outproj_chunks(prev_u[0], prev_u[1] * 8,
                                           prev_u[1] * 8 + 8)
                        )
                ub()
                prev_tail, prev_u = ut, u
            prev_tail()
            bg_step.extend(outproj_chunks(prev_u[0], prev_u[1] * 8,
                                          prev_u[1] * 8 + 8))
            drain_bound(len(bg_bound) + len(bg_step))
    _split_sync_waits(nc)
    return nc


def _get_nc():
    if "nc" not in _compiled:
        _compiled["nc"] = _build()
    return _compiled["nc"]


def _prep_in_maps(x, Wq, bq, Wk, bk, Wv, bv, Wo, bo):
    ET = D // P
    xf = np.ascontiguousarray(x.reshape(B * S, D))
    # x^T tiled: [p, et, token], e = et*128 + p
    xt = np.ascontiguousarray(
        xf.T.reshape(ET, P, B * S).transpose(1, 0, 2)
    ).astype(BF16)
    in_maps = []
    for c in range(NCORES):
        sl = slice(DSL * c, DSL * (c + 1))
        wqt = np.ascontiguousarray(
            Wq[sl].T.reshape(ET, P, DSL).transpose(1, 0, 2)
        ).astype(BF16)
        wkt = np.ascontiguousarray(
            Wk[sl].T.reshape(ET, P, DSL).transpose(1, 0, 2)
        ).astype(BF16)
        wvt = np.ascontiguousarray(
            Wv[sl].T.reshape(ET, P, DSL).transpose(1, 0, 2)
        ).astype(BF16)
        wot = np.ascontiguousarray(Wo[:, sl].T).astype(BF16)
        in_maps.append(
            {
                "xt": xt,
                "wq": wqt,
                "wk": wkt,
                "wv": wvt,
                "wo": wot,
                "bq": np.ascontiguousarray(bq[sl].reshape(P, 1)).astype(np.float32),
                "bk": np.ascontiguousarray(bk[sl].reshape(P, 1)).astype(np.float32),
            }
        )
    return in_maps


def kernel(x, Wq, bq, Wk, bk, Wv, bv, Wo, bo, _trace=False, _tmpdir=None):
    from concourse.bass_utils import run_bass_kernel_spmd

    x, Wq, bq, Wk, bk, Wv, bv, Wo, bo = (
        np.asarray(a, dtype=np.float32)
        for a in (x, Wq, bq, Wk, bk, Wv, bv, Wo, bo)
    )
    nc = _get_nc()
    in_maps = _prep_in_maps(x, Wq, bq, Wk, bk, Wv, bv, Wo, bo)
    res = run_bass_kernel_spmd(
        nc, in_maps, core_ids=list(range(NCORES)), trace=_trace, tmpdir=_tmpdir
    )
    total = np.zeros((2, B, S, D), np.float32)
    for c in range(NCORES):
        total += np.asarray(res.results[c]["out"], dtype=np.float32)
    const_vec = (bv @ Wo.T + bo).astype(np.float32)
    out = total[0] + const_vec
    out_comp = total[1] + const_vec
    if _trace:
        kernel._last_result = res
    return (out, out_comp)
